# revision 11
# baseline (speedup 1.0000x reference)
"""ArcFace (AngularPenaltySMLoss) on 8 TRN2 NeuronCores.

Sharding (model-parallel softmax): the 32768 classes are split across the
8 cores (4096 each).  Host prep is layout/dtype only: weights are scaled
by 64 (exact power-of-2 exponent shift, part of fp8 quantization - raw
xavier weights are subnormal in e4m3) and cast to fp8, features are cast
to fp8, and the target rows w[y_true] are gathered (pure indexing).

Per core (identical SPMD graph):
  - Weight-column norms: DVE squares (fp8) -> ones-matmul partition sums
    (PE, packed 4 chunks per PSUM z-slot at partitions 0/32/64/96) ->
    rsqrt as exp(-0.5*ln(ss/64)) on ACT -> K=1 matmul broadcast -> DVE
    mul to fp8 `what` (8*w_hat).  Chunks 0-3 use PSUM slot A, 4-7 slot B,
    so the main loop's first z-tile (slot B... slot A frees first) stalls
    minimally.
  - Features need no normalization pass: the matmul keeps batch on the
    partition axis, so 8/||f_b|| (from a DVE tensor_tensor_reduce of the
    fp8 features) becomes the per-partition `scale` of the main-loop Exp.
  - Main loop (32 z-tiles [128,2048]): z = fT^T @ what via fp8 DoubleRow
    matmuls; ACT Exp in place on PSUM with scale=8/||f|| and accum_out ->
    per-row partial exp sums.  ACT is the critical path (~2.1us/tile).
  - Per-row sums AllGather'd in two halves; the first hides under the
    second matmul sweep.
  - Target path on DVE tensor_tensor_reduce (fused mul+reduce), fully
    overlapped with the main loop; the margin transform needs only
    ln/exp (sqrt via exp(0.5*ln x)), all in the single activation table.
  - Tail after the last collective: fullsum add -> ln -> mean -> out.
"""
import math

import numpy as np
import ml_dtypes

import concourse.bass as bass
import concourse.tile as tile
from concourse import bacc, mybir
from concourse.bass_utils import run_bass_kernel_spmd

B = 2048          # batch
D = 512           # feature dim
C = 32768         # classes
NCORES = 8
CS = C // NCORES  # 4096 classes per core
S = 64.0
MARGIN = 0.5
EPS = 1e-7
COSM = math.cos(MARGIN)
SINM = math.sin(MARGIN)
WSCALE = 64.0     # host-side po2 quantization scale for fp8 weights

NB = B // 128     # 16 batch tiles
NK = D // 128     # 4 contraction chunks
NCC = CS // 512   # 8 class chunks per core

F32 = mybir.dt.float32
BF16 = mybir.dt.bfloat16
AF = mybir.ActivationFunctionType
ALU = mybir.AluOpType
FP8 = mybir.dt.float8e4
FP8NP = ml_dtypes.float8_e4m3fn

_CACHE = {}

_ONE_SET = "natural_log_exp_and_others"


def _patch_act_tables():
    from concourse import hw_specs, bacc as bacc_mod
    if getattr(bacc_mod, "_act_tables_patched", False):
        return
    orig = hw_specs.get_activation_tables

    def patched(arch):
        t = orig(arch)
        return {name: (funcs if name == _ONE_SET else set())
                for name, funcs in t.items()}

    bacc_mod.get_activation_tables = patched
    bacc_mod._act_tables_patched = True


def _build():
    _patch_act_tables()
    nc = bacc.Bacc(None, target_bir_lowering=False, debug=False)

    fT_ext = nc.declare_dram_parameter("fT", [D, B], FP8, isOutput=False)
    wT_ext = nc.declare_dram_parameter("wT", [D, CS], FP8, isOutput=False)
    fnat_ext = nc.declare_dram_parameter("fnat", [B, D], FP8, isOutput=False)
    wtgt_ext = nc.declare_dram_parameter("wtgt", [B, D], FP8, isOutput=False)
    out_ext = nc.declare_dram_parameter("out", [1, 1], F32, isOutput=True)

    cc_in = nc.dram_tensor("cc_in", [128, 2 * NB], F32)
    cc_out = nc.dram_tensor("cc_out", [128, 2 * NB], F32,
                            addr_space="Shared")

    with tile.TileContext(nc) as tc:
        with (
            tc.tile_pool(name="persist", bufs=1) as pp,
            tc.tile_pool(name="stream", bufs=4) as sp,
        ):
            # ---- persistent SBUF tiles ----
            wt3 = pp.tile([128, NK, CS], FP8)       # raw wT*64 (fp8)
            whats = [pp.tile([128, NK, 512], FP8, tag=f"what{i}",
                             name=f"what{i}")
                     for i in range(NCC)]           # normalized, per chunk
            ft3 = pp.tile([128, NK, B], FP8)        # raw fT (fp8) main lhsT
            fnat3 = pp.tile([128, NB, D], FP8)      # features, natural
            wtgt3 = pp.tile([128, NB, D], FP8)      # target weight rows
            ones_bf = pp.tile([128, 1], BF16)
            ones_f32 = pp.tile([128, 1], F32)
            ones_row = pp.tile([128, 128], BF16)  # sliced per 32-part base
            lrow_t = pp.tile([128, 512], F32)     # ln scratch, sliced
            rnr_t = pp.tile([128, 512], BF16)     # rsqrt rows, sliced
            sums = pp.tile([128, 2 * NB], F32)      # exp sums, (g, b)
            rs_pt = pp.tile([128, NB], F32)         # 8/||f_b|| exp scale
            rawdot = pp.tile([128, NB], F32)
            ssf = pp.tile([128, NB], F32)
            wn2 = pp.tile([128, NB], F32)

            nc.vector.memset(ones_bf[:], 1.0)
            nc.vector.memset(ones_f32[:], 1.0)
            nc.vector.memset(ones_row[:], 1.0)

            # ---- input DMA, spread across engine queues ----
            wTr = wT_ext[:].rearrange("(k p) c -> p k c", p=128)
            for n in range(4):
                nc.sync.dma_start(wt3[:, :, bass.ts(n, 512)],
                                  wTr[:, :, bass.ts(n, 512)])
            for n in range(4, NCC):
                nc.gpsimd.dma_start(wt3[:, :, bass.ts(n, 512)],
                                    wTr[:, :, bass.ts(n, 512)])
            fTr = fT_ext[:].rearrange("(k p) b -> p k b", p=128)
            nc.scalar.dma_start(ft3[:], fTr)
            fnr = fnat_ext[:].rearrange("(t p) d -> p t d", p=128)
            nc.scalar.dma_start(fnat3[:, 0:8, :], fnr[:, 0:8, :])
            nc.scalar.dma_start(fnat3[:, 8:16, :], fnr[:, 8:16, :])
            nc.gpsimd.dma_start(
                wtgt3[:], wtgt_ext[:].rearrange("(t p) d -> p t d", p=128))

            pmain_cm = tc.tile_pool(name="pmain", bufs=2, space="PSUM")
            pmain = pmain_cm.__enter__()

            # ---- weight-column norm prep, packed into two z slots ----
            # slot A holds chunks 0-3, slot B chunks 4-7: rowsums at
            # partitions 32*i (bank 0), bcasts rotate banks 1-3.
            zsA = pmain.tile([128, 2048], F32, tag="z", name="zsA")
            zsB = pmain.tile([128, 2048], F32, tag="z", name="zsB")

            def norm_chunk(n):
                zs = zsA if n < 4 else zsB
                i = n % 4
                p0 = 32 * i
                ps = zs[p0:p0 + 1, 0:512]
                for k in range(NK):
                    sq = sp.tile([128, 512], BF16, tag="sqt", name="sq")
                    nc.vector.tensor_mul(sq[:],
                                         wt3[:, k, bass.ts(n, 512)],
                                         wt3[:, k, bass.ts(n, 512)])
                    nc.tensor.matmul(ps, ones_bf[:], sq[:],
                                     start=(k == 0), stop=(k == NK - 1),
                                     tile_position=(0, p0))
                # rn = (ss/64)^-1/2 = 8/(64*||w||), via ln/exp (one table)
                lrow = lrow_t[p0:p0 + 1, :]
                nc.scalar.activation(lrow, ps, AF.Ln, scale=1.0 / 64.0)
                rnr = rnr_t[p0:p0 + 1, :]
                nc.scalar.activation(rnr, lrow, AF.Exp, scale=-0.5)
                pb = zs[:, 512 + 512 * (i % 3):1024 + 512 * (i % 3)]
                nc.tensor.matmul(pb, ones_row[p0:p0 + 1, :], rnr,
                                 start=True, stop=True,
                                 tile_position=(p0, 0))
                for k in range(NK):
                    nc.vector.tensor_mul(whats[n][:, k, :],
                                         wt3[:, k, bass.ts(n, 512)], pb)

            def row_dot(dst, t, src3a, src3b):
                """dst[:, t] = sum_d a[:,t,:]*b[:,t,:] (DVE mul + reduce)."""
                scr = sp.tile([128, D], BF16, tag="ttrs", name="scr")
                nc.vector.tensor_mul(scr[:], src3a[:, t, :], src3b[:, t, :])
                nc.vector.reduce_sum(dst[:, t:t + 1], scr[:],
                                     axis=mybir.AxisListType.X)

            for n in range(NCC):
                norm_chunk(n)
                # feature row norms, interleaved 2 per chunk (DVE)
                t0 = 2 * n
                for t in (t0, t0 + 1):
                    row_dot(ssf, t, fnat3, fnat3)

            # rs_pt = 8/||f|| = exp(-0.5*ln(ssf/64))
            lcol = pp.tile([128, NB], F32)
            nc.scalar.activation(lcol[:], ssf[:], AF.Ln, scale=1.0 / 64.0)
            nc.scalar.activation(rs_pt[:], lcol[:], AF.Exp, scale=-0.5)

            # ---- main loop, group 0 (classes 0-2047 of this shard) ----
            def sweep(g):
                for b in range(NB):
                    zp = pmain.tile([128, 2048], F32, tag="z", name="zp")
                    for c4 in range(4):
                        cc = 4 * g + c4
                        for j in range(NK // 2):
                            nc.tensor.matmul(
                                zp[:, bass.ts(c4, 512)],
                                ft3[:, 2 * j:2 * j + 2, bass.ts(b, 128)],
                                whats[cc][:, 2 * j:2 * j + 2, :],
                                start=(j == 0), stop=(j == 1),
                                perf_mode=mybir.MatmulPerfMode.DoubleRow)
                    nc.scalar.activation(
                        zp[:], zp[:], AF.Exp, scale=rs_pt[:, b:b + 1],
                        accum_out=sums[:, g * NB + b:g * NB + b + 1])

            sweep(0)

            # ---- target path (DVE mul+reduce, overlaps main loop) ----
            for t in range(NB):
                row_dot(rawdot, t, fnat3, wtgt3)
                row_dot(wn2, t, wtgt3, wtgt3)

            # ---- combine part 1 (no cross-core data needed) ----
            m2 = pp.tile([128, NB], F32)
            nc.vector.tensor_mul(m2[:], ssf[:], wn2[:])
            lm2 = pp.tile([128, NB], F32)
            nc.scalar.activation(lm2[:], m2[:], AF.Ln)
            rboth = pp.tile([128, NB], F32)
            nc.scalar.activation(rboth[:], lm2[:], AF.Exp, scale=-0.5)
            tgt = pp.tile([128, NB], F32)
            nc.vector.tensor_mul(tgt[:], rawdot[:], rboth[:])
            exptgt = pp.tile([128, NB], F32)
            nc.scalar.activation(exptgt[:], tgt[:], AF.Exp, scale=S)
            tclip = pp.tile([128, NB], F32)
            nc.vector.tensor_scalar(
                tclip[:], tgt[:], -1.0 + EPS, 1.0 - EPS,
                op0=ALU.max, op1=ALU.min)
            om = pp.tile([128, NB], F32)
            nc.vector.tensor_mul(om[:], tclip[:], tclip[:])
            nc.vector.tensor_scalar(om[:], om[:], -1.0, 1.0,
                                    op0=ALU.mult, op1=ALU.add)
            lom = pp.tile([128, NB], F32)
            nc.scalar.activation(lom[:], om[:], AF.Ln)
            snt = pp.tile([128, NB], F32)
            nc.scalar.activation(snt[:], lom[:], AF.Exp, scale=0.5)
            num = pp.tile([128, NB], F32)
            nc.vector.tensor_scalar_mul(num[:], tclip[:], S * COSM)
            snts = pp.tile([128, NB], F32)
            nc.vector.tensor_scalar_mul(snts[:], snt[:], S * SINM)
            nc.vector.tensor_sub(num[:], num[:], snts[:])
            expnum = pp.tile([128, NB], F32)
            nc.scalar.activation(expnum[:], num[:], AF.Exp)

            # ---- main loop, group 1 + tail collective ----
            sweep(1)
            nc.sync.dma_start(cc_in[:], sums[:])
            nc.gpsimd.collective_compute(
                "AllReduce", ALU.add,
                replica_groups=[list(range(NCORES))],
                ins=[cc_in[:].opt()],
                outs=[cc_out[:].opt()],
            )
            fsAll = pp.tile([128, 2 * NB], F32)
            nc.sync.dma_start(fsAll[:], cc_out[:])

            # ---- final combine ----
            denom = pp.tile([128, NB], F32)
            nc.vector.tensor_add(denom[:], fsAll[:, 0:NB], fsAll[:, NB:])
            nc.vector.tensor_add(denom[:], denom[:], expnum[:])
            nc.vector.tensor_sub(denom[:], denom[:], exptgt[:])
            logd = pp.tile([128, NB], F32)
            nc.scalar.activation(logd[:], denom[:], AF.Ln)
            lvals = pp.tile([128, NB], F32)
            nc.vector.tensor_sub(lvals[:], num[:], logd[:])
            lred = pp.tile([128, 1], F32)
            nc.vector.reduce_sum(lred[:], lvals[:], axis=mybir.AxisListType.X)
            zf = pmain.tile([128, 2048], F32, tag="z", name="zf")
            nc.tensor.matmul(zf[0:1, 0:1], ones_f32[:], lred[:],
                             start=True, stop=True)
            outv = pp.tile([1, 1], F32)
            nc.scalar.mul(outv[:], zf[0:1, 0:1], -1.0 / float(B))
            nc.sync.dma_start(out_ext[:], outv[:])
            pmain_cm.__exit__(None, None, None)

    nc.compile()
    return nc


def _prep_inputs(features, y_true, weight):
    features = np.asarray(features, dtype=np.float32)
    weight = np.asarray(weight, dtype=np.float32)
    y = np.asarray(y_true).astype(np.int64)

    f8 = features.astype(FP8NP)                        # [B, D]
    w8 = (weight * WSCALE).astype(FP8NP)               # [C, D] po2 quant
    fT = np.ascontiguousarray(f8.T)                    # [D, B]
    wtgt = w8[y]                                       # [B, D] gather

    in_maps = []
    for i in range(NCORES):
        wT = np.ascontiguousarray(w8[i * CS:(i + 1) * CS].T)  # [D, CS]
        in_maps.append({"fT": fT, "wT": wT, "fnat": f8, "wtgt": wtgt})
    return in_maps


def _run(features, y_true, weight, trace=False, **run_kwargs):
    if "nc" not in _CACHE:
        _CACHE["nc"] = _build()
    nc = _CACHE["nc"]
    in_maps = _prep_inputs(features, y_true, weight)
    res = run_bass_kernel_spmd(
        nc, in_maps, core_ids=list(range(NCORES)), trace=trace, **run_kwargs)
    out = np.asarray(res.results[0]["out"], dtype=np.float32)
    return np.float32(out.reshape(-1)[0]), res


def kernel(features, y_true, weight):
    val, _ = _run(features, y_true, weight, trace=False)
    return np.asarray(val, dtype=np.float32)
